# revision 17
# baseline (speedup 1.0000x reference)
"""Contrastive loss on Trainium2 (8 NeuronCores, SPMD, Bass/Tile).

Math
----
reference:
    norms[i,j] = ||x_i||^2 + ||x_j||^2 - 2 x_i.x_j
    pos = sum((eq - I) * norms) / cnt_pos          eq[i,j] = [y_i == y_j]
    neg = sum((1 - eq) * relu(1 - norms)) / cnt_neg
    loss = (pos + neg) / 2

pos is computed exactly on the host via the class-sum identity (O(N*D)).
The device computes the masked negative sum for the bulk of the pair
matrix; the host computes the thin remainder (diagonal-band blocks).

Device trick (fp8 DoubleRow, ONE matmul per output tile):
    u[i,j] = norms[i,j] - 1 + BIG * eq[i,j]          (BIG = 64 >= 1 + fp8 noise)

A single K=256 fp8 DoubleRow matmul packs both halves:
  - K-half 0 (128 rows): lhsT = -2 x_i^T, rhs = x_j^T          -> -2 G
  - K-half 1 (47 rows + zero pad): lhsT = [onehot; 1; sm1 hi/lo],
    rhs = [BIG*onehot; sq hi/lo; 1]                            -> BIG*eq + sq_j + (sq_i - 1)

Masked sums come out of u with ONE fused instruction per tile:
    ACT:  sum relu(-u)  = +sum_{eq=0} relu(1-norms)   (accum_out)
    DVE:  sum min(u,0)  = -sum_{eq=0} relu(1-norms)   (accum_out)
eq pairs land at u ~ d2-1+64 > 0 -> contribute 0.
fp8 margins (verified on data): min off-diag d2 ~ 121 >> 1, max value 205 < 240.

Work split (symmetry): with 128-row blocks r and 128-col blocks c (64 of
each), let d = (c - r) mod 64. Blocks d=1..31 are computed on device with
weight 2 (each unordered block pair visited once); d=0 and d=32 blocks
(1/33 of the pairs) are computed on the HOST (f32 GEMM over 128-row
blocks, exact eq masking) — this removes all small-tile device work, so
PSUM runs two clean 2048-col tiles (8 banks, double buffered) and the
ACT/DVE consume instruction count is minimal.

Sharding: core k owns global rows [1024k, 1024(k+1)). Its 8 row-blocks
need the circular column window [1024k+128, 1024k+4992) — the host ships
that window per-core ("rolled" columns), so the device program is
identical on every core (pure SPMD). Per-core outputs are per-partition
partial sums; the host reduces (O(N) work).
"""

import numpy as np
from contextlib import ExitStack

import concourse.bass as bass
import concourse.bacc as bacc
import concourse.tile as tile
from concourse import mybir
from concourse.bass_utils import run_bass_kernel_spmd

N, D, C = 8192, 128, 43
MARGIN = 1.0
BIG = 64.0
P = 128
NCORES = 8
ROWS_PER_CORE = N // NCORES           # 1024
RB = ROWS_PER_CORE // P               # 8 row-blocks per core
LOCAL_COLS = ROWS_PER_CORE + 30 * P   # 4864: cols [b+128, b+4096) for jj<8
AUGK = C + 4                          # 47 aug rows (onehot + sq hi/lo + ones)
WARMUP = 80                           # PE warm-up matmuls during DMA wait

# Per row-block jj (local col base b = 128*jj), device tiles:
#   T1: [b+128, b+2048)    1920 cols  (ACT)
#   T2: [b+2048, b+4096)   2048 cols  (DVE)
# All device columns have symmetry weight 2. Units: 2 per jj.
UNITS_PER_JJ = 2
NPART = UNITS_PER_JJ * RB
# sign: ACT computes +sum relu(1-d2); DVE computes -sum relu(1-d2)
UNIT_SIGN = np.array([+1.0, -1.0] * RB)

_cache = {}
TRACE = False


def _build_bass():
    f32 = mybir.dt.float32
    fp8 = mybir.dt.float8e4
    bf16 = mybir.dt.bfloat16
    nc = bacc.Bacc("TRN2", target_bir_lowering=False, debug=False)

    rx_d = nc.dram_tensor("rx", [P, 2, LOCAL_COLS], fp8, kind="ExternalInput").ap()
    wt_d = nc.dram_tensor("wt", [P, 2, ROWS_PER_CORE], fp8, kind="ExternalInput").ap()
    neg_out = nc.dram_tensor("neg_out", [P, NPART], f32, kind="ExternalOutput").ap()

    relu = mybir.ActivationFunctionType.Relu
    alu_min = mybir.AluOpType.min
    alu_add = mybir.AluOpType.add
    DR = mybir.MatmulPerfMode.DoubleRow

    with tile.TileContext(nc) as tc:
        with ExitStack() as ctx:
            const = ctx.enter_context(tc.tile_pool(name="const", bufs=1))
            psum = ctx.enter_context(tc.tile_pool(name="psum", bufs=2, space="PSUM"))
            scr_a = ctx.enter_context(tc.tile_pool(name="scr_a", bufs=2))
            scr_v = ctx.enter_context(tc.tile_pool(name="scr_v", bufs=2))

            # ---- constants / weights ----
            wu = const.tile([P, 32], fp8)          # warm-up weights
            nc.vector.memset(wu, 0.0)
            zbias = const.tile([P, 1], f32)
            nc.vector.memset(zbias, 0.0)
            negp = const.tile([P, NPART], f32)

            rxt = const.tile([P, 2, LOCAL_COLS], fp8)
            wt = const.tile([P, 2, ROWS_PER_CORE], fp8)

            # ---- input DMAs, in need order. aug half ships all 128 rows
            # (rows 47.. are zeros baked in DRAM: they meet zero weights,
            # but garbage NaN would poison 0*NaN). Early chunks on the sync
            # queue (scalar queue is blocked ~1.3us by ACT_TABLE_LOAD).
            nc.sync.dma_start(out=wt, in_=wt_d)
            c0, c1 = 0, 2048
            nc.sync.dma_start(out=rxt[:, 0:1, c0:c1], in_=rx_d[:, 0:1, c0:c1])
            nc.sync.dma_start(out=rxt[:, 1:2, c0:c1], in_=rx_d[:, 1:2, c0:c1])
            c0 = 2048
            nc.scalar.dma_start(out=rxt[:, 0:1, c0:], in_=rx_d[:, 0:1, c0:])
            nc.scalar.dma_start(out=rxt[:, 1:2, c0:], in_=rx_d[:, 1:2, c0:])

            # ---- PE warm-up during DMA wait (HAM un-throttle). FD=32 MMs
            # pipeline at ~27ns; keep the PE busy until data lands so the
            # HAM window flips to 2.4 GHz before the real matmuls.
            wps = psum.tile([P, 2048], f32, tag="ps")
            for _ in range(WARMUP):
                nc.tensor.matmul(wps[:32, 0:32], wu, wu, start=True, stop=True)

            def consume(t, ps, eng):
                fd = ps.shape[-1]
                if eng == "a":
                    sa = scr_a.tile([P, 2048], bf16, tag="sa")
                    nc.scalar.activation(sa[:, :fd], ps, relu, bias=zbias,
                                         scale=-1.0, accum_out=negp[:, t:t + 1])
                else:
                    sv = scr_v.tile([P, 2048], bf16, tag="sv")
                    nc.vector.tensor_scalar(sv[:, :fd], ps, 0.0, None, alu_min,
                                            op1=alu_add,
                                            accum_out=negp[:, t:t + 1])

            # ---- main loop: per row-block, 8 same-weight DoubleRow matmuls
            for jj in range(RB):
                b = jj * P
                u = UNITS_PER_JJ * jj
                wsl = wt[:, :, jj * P:(jj + 1) * P]

                t1 = psum.tile([P, 2048], f32, tag="ps")
                for q, w in enumerate((512, 512, 512, 384)):
                    c = b + q * 512
                    nc.tensor.matmul(t1[:, q * 512:q * 512 + w], wsl,
                                     rxt[:, :, c:c + w],
                                     start=True, stop=True, perf_mode=DR)
                consume(u + 0, t1[:, 0:1920], "a")

                t2 = psum.tile([P, 2048], f32, tag="ps")
                for q in range(4):
                    c = b + 1920 + q * 512
                    nc.tensor.matmul(t2[:, q * 512:(q + 1) * 512], wsl,
                                     rxt[:, :, c:c + 512],
                                     start=True, stop=True, perf_mode=DR)
                consume(u + 1, t2, "v")

            nc.sync.dma_start(out=neg_out, in_=negp)

    nc.compile()
    return nc


def _prep_inputs(x: np.ndarray, y: np.ndarray):
    """Host-side shard prep + thin-band blocks. O(N*D) + O(N*128*D)."""
    import ml_dtypes
    f8 = ml_dtypes.float8_e4m3fn

    x = np.ascontiguousarray(np.asarray(x, dtype=np.float32))
    y = np.asarray(y).astype(np.int64)
    assert x.shape == (N, D) and y.shape == (N,)

    # fp8-round x; derive sq from the ROUNDED x so device distance geometry
    # is self-consistent.
    x8 = x.astype(f8)
    xf = x8.astype(np.float32)
    sq = (xf * xf).sum(axis=1, dtype=np.float32)          # [N]
    oh = np.zeros((C, N), dtype=np.float32)
    oh[y, np.arange(N)] = 1.0

    xT8 = np.ascontiguousarray(x8.T)                      # [128, N] fp8

    def hi_lo(v):
        hi = v.astype(f8).astype(np.float32)
        lo = v - hi
        return hi, lo

    sq_hi, sq_lo = hi_lo(sq)
    sm1_hi, sm1_lo = hi_lo(sq - 1.0)

    aug_r = np.empty((AUGK, N), dtype=np.float32)
    aug_r[:C] = BIG * oh
    aug_r[C] = sq_hi
    aug_r[C + 1] = sq_lo
    aug_r[C + 2] = 1.0
    aug_r[C + 3] = 1.0
    aug_r = aug_r.astype(f8)

    aug_l = np.empty((AUGK, N), dtype=np.float32)
    aug_l[:C] = oh
    aug_l[C] = 1.0
    aug_l[C + 1] = 1.0
    aug_l[C + 2] = sm1_hi
    aug_l[C + 3] = sm1_lo
    aug_l = aug_l.astype(f8)

    m2xT = (-2.0 * xf.T).astype(f8)                       # [128, N], exact

    in_maps = []
    for k in range(NCORES):
        r0 = k * ROWS_PER_CORE
        # device covers cols [row_block + 128, row_block + 4096) per block:
        # global col window [r0 + 128, r0 + 128 + LOCAL_COLS)
        idx = (r0 + 128 + np.arange(LOCAL_COLS)) % N
        rows = slice(r0, r0 + ROWS_PER_CORE)

        rx = np.zeros((P, 2, LOCAL_COLS), dtype=f8)
        rx[:, 0, :] = xT8[:, idx]
        rx[:AUGK, 1, :] = aug_r[:, idx]

        wt = np.zeros((P, 2, ROWS_PER_CORE), dtype=f8)
        wt[:, 0, :] = m2xT[:, rows]
        wt[:AUGK, 1, :] = aug_l[:, rows]

        in_maps.append({"rx": rx, "wt": wt})

    cnt = np.bincount(y, minlength=C).astype(np.float64)
    sum_sq_cnt = float((cnt * cnt).sum())
    pos_cnt = sum_sq_cnt - N
    neg_cnt = float(N) * N - sum_sq_cnt

    # pos term via the O(N*D) identity, f64 on the ORIGINAL f32 x.
    x64 = x.astype(np.float64)
    sq64 = (x64 * x64).sum(axis=1)
    S = np.zeros((C, D), dtype=np.float64)
    np.add.at(S, y, x64)
    pos_sum = 2.0 * float((sq64 * cnt[y]).sum()) - 2.0 * float((S * S).sum())

    # ---- host neg contribution of the d=0 and d=32 block bands (the thin
    # remainder not covered on device). Same fp8-rounded geometry as the
    # device; exact eq masking (excludes diagonal and same-label pairs).
    XB = xf.reshape(64, P, D)                             # 64 row-blocks
    sqB = sq.reshape(64, P)
    yB = y.reshape(64, P)
    host_neg = 0.0
    for dblk in (0, 32):
        XR = np.roll(XB, -dblk, axis=0)                   # col block r+d
        G = np.einsum('bij,bkj->bik', XB, XR, optimize=True)
        d2 = sqB[:, :, None] + np.roll(sqB, -dblk, axis=0)[:, None, :] - 2.0 * G
        neqm = yB[:, :, None] != np.roll(yB, -dblk, axis=0)[:, None, :]
        host_neg += float(np.maximum(MARGIN - d2, 0.0)[neqm].sum())
    # d=0 band counted once; d=32 band: rolling covers both mirror
    # orderings across the 64 blocks (block r pairs with r+32 for all r),
    # which matches the weight-1 "both copies visited" convention.
    return in_maps, pos_cnt, neg_cnt, pos_sum, host_neg


def _reduce_outputs(results, host_neg):
    neg_sum = host_neg
    for r in results:
        # device blocks all carry symmetry weight 2
        neg_sum += 2.0 * float((r["neg_out"].astype(np.float64).sum(axis=0)
                                * UNIT_SIGN).sum())
    return neg_sum


def kernel(x: np.ndarray, y: np.ndarray) -> np.ndarray:
    in_maps, pos_cnt, neg_cnt, pos_sum, host_neg = _prep_inputs(x, y)

    if "nc" not in _cache:
        _cache["nc"] = _build_bass()
    nc = _cache["nc"]

    res = run_bass_kernel_spmd(nc, in_maps, core_ids=list(range(NCORES)),
                               trace=TRACE)
    _cache["last_results"] = res

    neg_sum = _reduce_outputs(res.results, host_neg)
    loss = (pos_sum / pos_cnt + neg_sum / neg_cnt) / 2.0
    return np.float32(loss)


# revision 21
# speedup vs baseline: 1.3322x; 1.3322x over previous
"""Contrastive loss on Trainium2 (8 NeuronCores, SPMD, Bass/Tile).

Math
----
reference:
    norms[i,j] = ||x_i||^2 + ||x_j||^2 - 2 x_i.x_j
    pos = sum((eq - I) * norms) / cnt_pos          eq[i,j] = [y_i == y_j]
    neg = sum((1 - eq) * relu(1 - norms)) / cnt_neg
    loss = (pos + neg) / 2

pos is computed exactly on the host via the class-sum identity (O(N*D)).
The device computes the masked negative sum for the bulk of the pair
matrix; the host computes the thin remainder (diagonal-band blocks).

Device trick (fp8 DoubleRow, ONE matmul per output tile):
    u[i,j] = norms[i,j] - 1 + BIG * eq[i,j]          (BIG = 64 >= 1 + fp8 noise)

A single K=256 fp8 DoubleRow matmul packs both halves:
  - K-half 0 (128 rows): lhsT = -2 x_i^T, rhs = x_j^T          -> -2 G
  - K-half 1 (47 rows + zero pad): lhsT = [onehot; 1; sm1 hi/lo],
    rhs = [BIG*onehot; sq hi/lo; 1]                            -> BIG*eq + sq_j + (sq_i - 1)

Masked sums come out of u with ONE fused instruction per tile:
    ACT:  sum relu(-u)  = +sum_{eq=0} relu(1-norms)   (accum_out)
    DVE:  sum min(u,0)  = -sum_{eq=0} relu(1-norms)   (accum_out)
eq pairs land at u ~ d2-1+64 > 0 -> contribute 0.
fp8 margins (verified on data): min off-diag d2 ~ 121 >> 1, max value 205 < 240.

Work split (symmetry): with 128-row blocks r and 128-col blocks c (64 of
each), let d = (c - r) mod 64. Blocks d=1..31 are computed on device with
weight 2 (each unordered block pair visited once); d=0 and d=32 blocks
(1/33 of the pairs) are computed on the HOST (f32 GEMM over 128-row
blocks, exact eq masking) — this removes all small-tile device work, so
PSUM runs two clean 2048-col tiles (8 banks, double buffered) and the
ACT/DVE consume instruction count is minimal.

Sharding: core k owns global rows [1024k, 1024(k+1)). Its 8 row-blocks
need the circular column window [1024k+128, 1024k+4992) — the host ships
that window per-core ("rolled" columns), so the device program is
identical on every core (pure SPMD). Per-core outputs are per-partition
partial sums; the host reduces (O(N) work).
"""

import numpy as np
from contextlib import ExitStack

import concourse.bass as bass
import concourse.bacc as bacc
import concourse.tile as tile
from concourse import mybir
from concourse.bass_utils import run_bass_kernel_spmd

N, D, C = 8192, 128, 43
MARGIN = 1.0
BIG = 64.0
P = 128
NCORES = 8
ROWS_PER_CORE = N // NCORES           # 1024
RB = ROWS_PER_CORE // P               # 8 row-blocks per core
LOCAL_COLS = ROWS_PER_CORE + 30 * P   # 4864: cols [b+128, b+4096) for jj<8
AUGK = C + 4                          # 47 aug rows (onehot + sq hi/lo + ones)
WARMUP = 80                           # PE warm-up matmuls during DMA wait

# Per row-block jj (local col base b = 128*jj), device tiles (1024 cols,
# alternating consumers so neither engine's latency chains the pipeline):
#   m0 [b, b+1024) ACT | m1 [b+1024, b+2048) DVE
#   m2 [b+2048, b+3072) ACT | m3 [b+3072, b+3968) DVE (896)
# All device columns have symmetry weight 2. Units: 4 per jj.
UNITS_PER_JJ = 4
NPART = UNITS_PER_JJ * RB
# sign: ACT computes +sum relu(1-d2); DVE computes -sum relu(1-d2)
UNIT_SIGN = np.array([+1.0, -1.0, +1.0, -1.0] * RB)

_cache = {}
TRACE = False


def _build_bass():
    f32 = mybir.dt.float32
    fp8 = mybir.dt.float8e4
    bf16 = mybir.dt.bfloat16
    nc = bacc.Bacc("TRN2", target_bir_lowering=False, debug=False)

    rx_d = nc.dram_tensor("rx", [P, 2, LOCAL_COLS], fp8, kind="ExternalInput").ap()
    wt_d = nc.dram_tensor("wt", [P, 2, ROWS_PER_CORE], fp8, kind="ExternalInput").ap()
    neg_out = nc.dram_tensor("neg_out", [P, NPART], f32, kind="ExternalOutput").ap()

    relu = mybir.ActivationFunctionType.Relu
    alu_min = mybir.AluOpType.min
    alu_add = mybir.AluOpType.add
    DR = mybir.MatmulPerfMode.DoubleRow

    with tile.TileContext(nc) as tc:
        with ExitStack() as ctx:
            const = ctx.enter_context(tc.tile_pool(name="const", bufs=1))
            psum = ctx.enter_context(tc.tile_pool(name="psum", bufs=4, space="PSUM"))
            scr_a = ctx.enter_context(tc.tile_pool(name="scr_a", bufs=2))
            scr_v = ctx.enter_context(tc.tile_pool(name="scr_v", bufs=2))

            # ---- constants / weights ----
            wu = const.tile([P, 32], fp8)          # warm-up weights
            nc.vector.memset(wu, 0.0)
            zbias = const.tile([P, 1], f32)
            nc.vector.memset(zbias, 0.0)
            negp = const.tile([P, NPART], f32)

            rxt = const.tile([P, 2, LOCAL_COLS], fp8)
            wt = const.tile([P, 2, ROWS_PER_CORE], fp8)

            # ---- input DMAs, in need order. aug half ships all 128 rows
            # (rows 47.. are zeros baked in DRAM: they meet zero weights,
            # but garbage NaN would poison 0*NaN). Early chunks on the sync
            # queue (scalar queue is blocked ~1.3us by ACT_TABLE_LOAD).
            nc.sync.dma_start(out=wt, in_=wt_d)
            c0, c1 = 0, 1024
            nc.sync.dma_start(out=rxt[:, 0:1, c0:c1], in_=rx_d[:, 0:1, c0:c1])
            nc.sync.dma_start(out=rxt[:, 1:2, c0:c1], in_=rx_d[:, 1:2, c0:c1])
            c0, c1 = 1024, 2944
            nc.scalar.dma_start(out=rxt[:, 0:1, c0:c1], in_=rx_d[:, 0:1, c0:c1])
            nc.scalar.dma_start(out=rxt[:, 1:2, c0:c1], in_=rx_d[:, 1:2, c0:c1])
            c0 = 2944
            nc.sync.dma_start(out=rxt[:, 0:1, c0:], in_=rx_d[:, 0:1, c0:])
            nc.sync.dma_start(out=rxt[:, 1:2, c0:], in_=rx_d[:, 1:2, c0:])

            # ---- PE warm-up during DMA wait (HAM un-throttle). FD=32 MMs
            # pipeline at ~27ns; keep the PE busy until data lands so the
            # HAM window flips to 2.4 GHz before the real matmuls.
            wps = psum.tile([P, 1024], f32, tag="ps")
            for _ in range(WARMUP):
                nc.tensor.matmul(wps[:32, 0:32], wu, wu, start=True, stop=True)

            def consume(t, ps, eng):
                fd = ps.shape[-1]
                if eng == "a":
                    sa = scr_a.tile([P, 1024], bf16, tag="sa")
                    nc.scalar.activation(sa[:, :fd], ps, relu, bias=zbias,
                                         scale=-1.0, accum_out=negp[:, t:t + 1])
                else:
                    sv = scr_v.tile([P, 1024], bf16, tag="sv")
                    nc.vector.tensor_scalar(sv[:, :fd], ps, 0.0, None, alu_min,
                                            op1=alu_add,
                                            accum_out=negp[:, t:t + 1])

            # ---- main loop: per row-block, 8 same-weight DoubleRow matmuls
            for jj in range(RB):
                b = jj * P
                u = UNITS_PER_JJ * jj
                wsl = wt[:, :, jj * P:(jj + 1) * P]

                for m in range(4):
                    c0 = b + m * 1024
                    fd = 1024 if m < 3 else 896
                    mt = psum.tile([P, 1024], f32, tag="ps")
                    nc.tensor.matmul(mt[:, 0:512], wsl, rxt[:, :, c0:c0 + 512],
                                     start=True, stop=True, perf_mode=DR)
                    nc.tensor.matmul(mt[:, 512:fd], wsl,
                                     rxt[:, :, c0 + 512:c0 + fd],
                                     start=True, stop=True, perf_mode=DR)
                    consume(u + m, mt[:, :fd], "a" if m % 2 == 0 else "v")

            nc.sync.dma_start(out=neg_out, in_=negp)

    nc.compile()
    return nc


def _prep_inputs(x: np.ndarray, y: np.ndarray):
    """Host-side shard prep + thin-band blocks. O(N*D) + O(N*128*D)."""
    import ml_dtypes
    f8 = ml_dtypes.float8_e4m3fn

    x = np.ascontiguousarray(np.asarray(x, dtype=np.float32))
    y = np.asarray(y).astype(np.int64)
    assert x.shape == (N, D) and y.shape == (N,)

    # fp8-round x; derive sq from the ROUNDED x so device distance geometry
    # is self-consistent.
    x8 = x.astype(f8)
    xf = x8.astype(np.float32)
    sq = (xf * xf).sum(axis=1, dtype=np.float32)          # [N]
    oh = np.zeros((C, N), dtype=np.float32)
    oh[y, np.arange(N)] = 1.0

    xT8 = np.ascontiguousarray(x8.T)                      # [128, N] fp8

    def hi_lo(v):
        hi = v.astype(f8).astype(np.float32)
        lo = v - hi
        return hi, lo

    sq_hi, sq_lo = hi_lo(sq)
    sm1_hi, sm1_lo = hi_lo(sq - 1.0)

    aug_r = np.empty((AUGK, N), dtype=np.float32)
    aug_r[:C] = BIG * oh
    aug_r[C] = sq_hi
    aug_r[C + 1] = sq_lo
    aug_r[C + 2] = 1.0
    aug_r[C + 3] = 1.0
    aug_r = aug_r.astype(f8)

    aug_l = np.empty((AUGK, N), dtype=np.float32)
    aug_l[:C] = oh
    aug_l[C] = 1.0
    aug_l[C + 1] = 1.0
    aug_l[C + 2] = sm1_hi
    aug_l[C + 3] = sm1_lo
    aug_l = aug_l.astype(f8)

    m2xT = (-2.0 * xf.T).astype(f8)                       # [128, N], exact

    in_maps = []
    for k in range(NCORES):
        r0 = k * ROWS_PER_CORE
        # device covers cols [row_block + 128, row_block + 4096) per block:
        # global col window [r0 + 128, r0 + 128 + LOCAL_COLS)
        idx = (r0 + 128 + np.arange(LOCAL_COLS)) % N
        rows = slice(r0, r0 + ROWS_PER_CORE)

        rx = np.zeros((P, 2, LOCAL_COLS), dtype=f8)
        rx[:, 0, :] = xT8[:, idx]
        rx[:AUGK, 1, :] = aug_r[:, idx]

        wt = np.zeros((P, 2, ROWS_PER_CORE), dtype=f8)
        wt[:, 0, :] = m2xT[:, rows]
        wt[:AUGK, 1, :] = aug_l[:, rows]

        in_maps.append({"rx": rx, "wt": wt})

    cnt = np.bincount(y, minlength=C).astype(np.float64)
    sum_sq_cnt = float((cnt * cnt).sum())
    pos_cnt = sum_sq_cnt - N
    neg_cnt = float(N) * N - sum_sq_cnt

    # pos term via the O(N*D) identity, f64 on the ORIGINAL f32 x.
    x64 = x.astype(np.float64)
    sq64 = (x64 * x64).sum(axis=1)
    S = np.zeros((C, D), dtype=np.float64)
    np.add.at(S, y, x64)
    pos_sum = 2.0 * float((sq64 * cnt[y]).sum()) - 2.0 * float((S * S).sum())

    # ---- host neg contribution of the d=0 and d=32 block bands (the thin
    # remainder not covered on device). Same fp8-rounded geometry as the
    # device; exact eq masking (excludes diagonal and same-label pairs).
    XB = xf.reshape(64, P, D)                             # 64 row-blocks
    sqB = sq.reshape(64, P)
    yB = y.reshape(64, P)
    host_neg = 0.0
    for dblk in (0, 32):
        XR = np.roll(XB, -dblk, axis=0)                   # col block r+d
        G = np.einsum('bij,bkj->bik', XB, XR, optimize=True)
        d2 = sqB[:, :, None] + np.roll(sqB, -dblk, axis=0)[:, None, :] - 2.0 * G
        neqm = yB[:, :, None] != np.roll(yB, -dblk, axis=0)[:, None, :]
        host_neg += float(np.maximum(MARGIN - d2, 0.0)[neqm].sum())
    # d=0 band counted once; d=32 band: rolling covers both mirror
    # orderings across the 64 blocks (block r pairs with r+32 for all r),
    # which matches the weight-1 "both copies visited" convention.
    return in_maps, pos_cnt, neg_cnt, pos_sum, host_neg


def _reduce_outputs(results, host_neg):
    neg_sum = host_neg
    for r in results:
        # device blocks all carry symmetry weight 2
        neg_sum += 2.0 * float((r["neg_out"].astype(np.float64).sum(axis=0)
                                * UNIT_SIGN).sum())
    return neg_sum


def kernel(x: np.ndarray, y: np.ndarray) -> np.ndarray:
    in_maps, pos_cnt, neg_cnt, pos_sum, host_neg = _prep_inputs(x, y)

    if "nc" not in _cache:
        _cache["nc"] = _build_bass()
    nc = _cache["nc"]

    res = run_bass_kernel_spmd(nc, in_maps, core_ids=list(range(NCORES)),
                               trace=TRACE)
    _cache["last_results"] = res

    neg_sum = _reduce_outputs(res.results, host_neg)
    loss = (pos_sum / pos_cnt + neg_sum / neg_cnt) / 2.0
    return np.float32(loss)
